# revision 1
# baseline (speedup 1.0000x reference)
"""Trainium2 Bass kernel for nn_AttentionBlock (causal single-head attention,
8192 tokens, qk-dim 16, v-dim 128, 1x1-conv projections with positional enc).

Sharding: striped query-parallel over 8 cores. Core m owns query tokens
{m, m+8, ..., m+8184} (1024 queries) — perfectly balanced causal work AND an
identical instruction stream on every core (required: one NEFF, SPMD). All
per-core variation is input data: the core's Q stripe and a small causal mask.

Device dataflow per core (fp32 data, float32r matmul streaming mode):
  - K^T [16, 8192] and Q^T [16, 1024] via matmuls (channels contracted,
    padded to 3x128 with bias folded in via a ones-channel; the 1/sqrt(259)
    score scale is folded into Wq/bq host-side).
  - V_aug [8192, 129] (tokens on partitions) via matmuls with x stationary;
    column 128 is all-ones (from the ones-channel) -> softmax denominators
    come free from the attn@V matmul. Weight padded to 256 cols so the
    matmul free dim stays >= 256 (float32r full-rate threshold).
  - Queries processed in PAIRS of 128-query subblocks: S^T tiles
    [128 keys, 256 q] (K stationary), exp on ScalarE (PSUM->SBUF),
    multiplicative causal mask on the 16 diagonal key blocks of each pair
    (which also zeroes the below-diagonal region), then attn@V accumulation
    o[128 q, 256] (cols >=129 are don't-care garbage read from the flat V
    buffer to keep the free dim >= 256) with A^T stationary.
  - Normalize with per-partition scale 1/(sum+1e-30) on ScalarE; DMA out as
    [1024 q, 128 vc]; host reassembles/transposes stripes.
"""

import os
import numpy as np

P = 128
NTOK = 8192
KC, VC = 16, 128
NCORES = 8
NQ = NTOK // NCORES       # 1024 queries per core
QSUBS = NQ // P           # 8
NPAIR = QSUBS // 2        # 4 query-subblock pairs
VW = VC + 1               # 129: V columns + ones column
VSTR = 132                # V row stride in SBUF (128 v + 1 ones + pad)
CPAD = 264                # channels: 259 x + 3 pos + 1 ones + 1 zero pad
CTAIL = CPAD - 256        # rows in the third (partial) channel chunk
NT_CHUNKS = NTOK // 512   # 16 token chunks for K-proj / xkv DMA

LAST_RESULTS = None       # BassKernelResults of the most recent run (for test.py)

_CACHE = {}


def _build_bass():
    import concourse.mybir as mybir
    import concourse.tile as tile
    from concourse import bacc

    f32 = mybir.dt.float32
    f32r = mybir.dt.float32r
    bf16 = mybir.dt.bfloat16
    AF = mybir.ActivationFunctionType

    nc = bacc.Bacc("TRN2", target_bir_lowering=False, debug=False,
                   num_devices=NCORES)

    xq_d = nc.dram_tensor("xq", [CPAD, NQ], f32r, kind="ExternalInput").ap()
    xkv_d = nc.dram_tensor("xkv", [CPAD, NTOK], f32r, kind="ExternalInput").ap()
    ww_d = nc.dram_tensor("ww", [CPAD, 288], f32r, kind="ExternalInput").ap()
    mask_d = nc.dram_tensor("mask", [P, 16 * 256], bf16, kind="ExternalInput").ap()
    y_d = nc.dram_tensor("y", [NQ, VC], f32, kind="ExternalOutput").ap()
    dbg = bool(int(os.environ.get("KDBG", "0")))
    if dbg:
        ktd = nc.dram_tensor("ktd", [KC, NTOK], f32, kind="ExternalOutput").ap()
        qtd = nc.dram_tensor("qtd", [KC, NQ], f32, kind="ExternalOutput").ap()
        vd = nc.dram_tensor("vd", [P, 4 * VSTR], f32, kind="ExternalOutput").ap()

    with tile.TileContext(nc) as tc:
        with (
            tc.tile_pool(name="const", bufs=1) as const,
            tc.tile_pool(name="xpool", bufs=8) as xpool,
            tc.tile_pool(name="work", bufs=6) as work,
            tc.tile_pool(name="small", bufs=8) as small,
            tc.tile_pool(name="ps_s", bufs=3, space="PSUM") as ps_s,
            tc.tile_pool(name="ps_olo", bufs=1, space="PSUM") as ps_olo,
            tc.tile_pool(name="ps_ohi", bufs=1, space="PSUM") as ps_ohi,
            tc.tile_pool(name="ps_kq", bufs=1, space="PSUM") as ps_kq,
            tc.tile_pool(name="ps_v", bufs=2, space="PSUM") as ps_v,
        ):
          # ---- body (emitted KREPEAT times for device-time measurement) ----
          for _rep in range(int(os.environ.get("KREPEAT", "1"))):
            # ---- persistent SBUF tensors ----
            ww_sb = const.tile([P, 3, 288], f32r)
            mask_sb = const.tile([P, 16 * 256], bf16)
            xq_sb = const.tile([P, 3, NQ], f32r)
            qt_sb = const.tile([KC, NQ], f32r)
            kt_sb = const.tile([KC, NTOK], f32r)
            v_sb = const.tile([P, (NTOK // P) * VSTR], f32r)
            # pad columns (VW..VSTR of each block) are streamed by the
            # 256-wide attn@V reads; give them defined values once
            nc.gpsimd.memset(v_sb[:].bitcast(f32), 0.0)

            # ---- DMA ordering: packed weights first (one instruction),
            # then the bytes that unblock pair-0 compute, in dep order ----
            nc.sync.dma_start(ww_sb[:, 0:2, :],
                              ww_d[0:256].rearrange("(c p) m -> p c m", p=P))
            nc.sync.dma_start(ww_sb[0:CTAIL, 2, :], ww_d[256:CPAD])
            wq_sb = ww_sb[:, :, 0:KC]
            wk_sb = ww_sb[:, :, KC:2 * KC]
            wv_sb = ww_sb[:, :, 2 * KC:288]
            chunk_tiles = {}

            def ensure_chunk_dma(nt):
                if nt in chunk_tiles or nt >= NT_CHUNKS:
                    return
                sl = slice(nt * 512, (nt + 1) * 512)
                xc_sb = xpool.tile([P, 3, 512], f32r, tag="xc", name=f"xc{nt}")
                nc.sync.dma_start(
                    xc_sb[:, 0:2, :],
                    xkv_d[0:256, sl].rearrange("(c p) n -> p c n", p=P))
                nc.sync.dma_start(xc_sb[0:CTAIL, 2, :], xkv_d[256:CPAD, sl])
                chunk_tiles[nt] = xc_sb

            ensure_chunk_dma(0)
            nc.sync.dma_start(
                xq_sb[:, 0:2, :],
                xq_d[0:256].rearrange("(c p) n -> p c n", p=P))
            nc.sync.dma_start(xq_sb[0:CTAIL, 2, :], xq_d[256:CPAD])
            ensure_chunk_dma(1)
            nc.sync.dma_start(mask_sb[:], mask_d)

            # ---- Q projection: QT [16, 1024] ----
            for h in range(2):
                q_ps = ps_kq.tile([KC, 512], f32, tag="kq")
                for ci in range(3):
                    kk = P if ci < 2 else CTAIL
                    nc.tensor.matmul(
                        q_ps[:], wq_sb[0:kk, ci, :],
                        xq_sb[0:kk, ci, h * 512:(h + 1) * 512],
                        start=(ci == 0), stop=(ci == 2),
                    )
                nc.vector.tensor_copy(qt_sb[:, h * 512:(h + 1) * 512], q_ps[:])

            def emit_chunk(nt):
                """Project K and V from a DMA'd 512-token chunk of xkv."""
                sl = slice(nt * 512, (nt + 1) * 512)
                ensure_chunk_dma(nt)
                xc_sb = chunk_tiles.pop(nt)
                k_ps = ps_kq.tile([KC, 512], f32, tag="kq")
                for ci in range(3):
                    kk = P if ci < 2 else CTAIL
                    nc.tensor.matmul(
                        k_ps[:], wk_sb[0:kk, ci, :], xc_sb[0:kk, ci, :],
                        start=(ci == 0), stop=(ci == 2),
                    )
                nc.vector.tensor_copy(kt_sb[:, sl], k_ps[:])
                if dbg:
                    nc.sync.dma_start(ktd[:, sl], kt_sb[:, sl].bitcast(f32))
                for tl in range(4):
                    tb = 4 * nt + tl
                    v_ps = ps_v.tile([P, 256], f32, tag="v")
                    for ci in range(3):
                        kk = P if ci < 2 else CTAIL
                        nc.tensor.matmul(
                            v_ps[:], xc_sb[0:kk, ci, tl * P:(tl + 1) * P],
                            wv_sb[0:kk, ci, :],
                            start=(ci == 0), stop=(ci == 2),
                        )
                    nc.vector.tensor_copy(
                        v_sb[:, tb * VSTR: tb * VSTR + VW], v_ps[:, :VW])
                    if dbg and tb < 4:
                        nc.sync.dma_start(
                            vd[:, tb * VSTR: tb * VSTR + VW],
                            v_sb[:, tb * VSTR: tb * VSTR + VW].bitcast(f32))

            if dbg:
                nc.sync.dma_start(qtd, qt_sb[:].bitcast(f32))
            # ---- main loop over query-subblock pairs ----
            for p in range(NPAIR):
                for nt in range(4 * p, 4 * p + 4):
                    emit_chunk(nt)
                    ensure_chunk_dma(nt + 2)  # stay 2 chunks ahead

                nkb = 16 * p + 16          # key blocks for this pair
                ngr = nkb // 2             # S-tile groups (2 kbs each)
                o_lo = ps_olo.tile([P, 256], f32, tag="olo")
                o_hi = ps_ohi.tile([P, 256], f32, tag="ohi")
                nmm_lo = 16 * p + 8
                nmm_hi = nkb
                qcol = slice(2 * p * P, 2 * p * P + 256)
                for g in range(ngr):
                    s_ps = ps_s.tile([P, 512], f32, tag="s")
                    for u in range(2):
                        kb = 2 * g + u
                        nc.tensor.matmul(
                            s_ps[:, u * 256:(u + 1) * 256],
                            kt_sb[:, kb * P:(kb + 1) * P],
                            qt_sb[:, qcol],
                            start=True, stop=True,
                        )
                    a_sb = work.tile([P, 512], f32r, tag="a")
                    nc.scalar.activation(a_sb[:], s_ps[:], AF.Exp)
                    if g >= ngr - 8:
                        dloc = g - (ngr - 8)
                        nc.vector.tensor_mul(
                            a_sb[:], a_sb[:],
                            mask_sb[:, dloc * 512:(dloc + 1) * 512])
                    for u in range(2):
                        kb = 2 * g + u
                        # attn@V for the lower qsub of the pair (only kbs
                        # strictly below its diagonal region contribute)
                        if kb < nmm_lo:
                            nc.tensor.matmul(
                                o_lo[:],
                                a_sb[:, u * 256: u * 256 + P],
                                v_sb[:, kb * VSTR: kb * VSTR + 256],
                                start=(kb == 0), stop=(kb == nmm_lo - 1),
                            )
                        # upper qsub: all kbs; last kb can't garbage-read past
                        # its own V rows (next block not written yet), narrow N
                        # (132 not 129: fp32r matmuls reject odd free dims)
                        w = 256 if kb < nkb - 1 else VSTR
                        nc.tensor.matmul(
                            o_hi[:, :w],
                            a_sb[:, u * 256 + P: u * 256 + 256],
                            v_sb[:, kb * VSTR: kb * VSTR + w],
                            start=(kb == 0), stop=(kb == nmm_hi - 1),
                        )

                for half, o_acc in ((0, o_lo), (1, o_hi)):
                    qs = 2 * p + half
                    recip_sb = small.tile([P, 1], f32, tag="recip")
                    nc.vector.tensor_scalar_add(recip_sb[:], o_acc[:, VC:VC + 1],
                                                1e-30)
                    nc.vector.reciprocal(recip_sb[:], recip_sb[:])
                    on_sb = small.tile([P, VC], f32, tag="on")
                    nc.scalar.activation(on_sb[:], o_acc[:, 0:VC], AF.Copy,
                                         scale=recip_sb[:])
                    nc.sync.dma_start(y_d[qs * P:(qs + 1) * P, :], on_sb[:])

    nc.compile()
    return nc


def _host_prep(x, Wq, bq, Wk, bk, Wv, bv):
    x = np.ascontiguousarray(np.asarray(x, np.float32))
    xc = np.zeros((CPAD, NTOK), np.float32)
    xc[:259] = x.reshape(259, NTOK)
    t = np.arange(8, dtype=np.float32) / 8 - 0.5
    h = np.arange(32, dtype=np.float32) / 32 - 0.5
    w = np.arange(32, dtype=np.float32) / 32 - 0.5
    pe = np.zeros((3, 8, 32, 32), np.float32)
    pe[0] = t[:, None, None]
    pe[1] = h[None, :, None]
    pe[2] = w[None, None, :]
    xc[259:262] = pe.reshape(3, NTOK)
    xc[262] = 1.0
    s = np.float32(1.0) / np.sqrt(np.float32(259.0))
    ww = np.zeros((CPAD, 288), np.float32)
    ww[:262, 0:KC] = np.asarray(Wq, np.float32).T * s
    ww[262, 0:KC] = np.asarray(bq, np.float32) * s
    ww[:262, KC:2 * KC] = np.asarray(Wk, np.float32).T
    ww[262, KC:2 * KC] = np.asarray(bk, np.float32)
    ww[:262, 2 * KC:2 * KC + VC] = np.asarray(Wv, np.float32).T
    ww[262, 2 * KC:2 * KC + VC] = np.asarray(bv, np.float32)
    ww[262, 2 * KC + VC] = 1.0
    return xc, ww


def _core_mask(m):
    # mask[jp, i*256 + tt2] = 1 iff 8*tt2 + m > 128*i + jp, i in 0..15
    jp = np.arange(P)[:, None]
    col = np.arange(16 * 256)[None, :]
    i = col // 256
    tt2 = col % 256
    import ml_dtypes
    return np.ascontiguousarray(
        (8 * tt2 + m > 128 * i + jp).astype(ml_dtypes.bfloat16))


def kernel(x, Wq, bq, Wk, bk, Wv, bv):
    global LAST_RESULTS
    from concourse.bass_utils import run_bass_kernel_spmd

    if "nc" not in _CACHE:
        _CACHE["nc"] = _build_bass()
    nc = _CACHE["nc"]

    xc, ww = _host_prep(x, Wq, bq, Wk, bk, Wv, bv)
    in_maps = []
    for m in range(NCORES):
        in_maps.append({
            "xq": np.ascontiguousarray(xc[:, m::8]),
            "xkv": xc,
            "ww": ww,
            "mask": _core_mask(m),
        })

    res = run_bass_kernel_spmd(
        nc, in_maps, core_ids=list(range(NCORES)),
        trace=bool(int(os.environ.get("KBENCH_TRACE", "0"))),
    )
    LAST_RESULTS = res

    out = np.zeros((VC, NQ, NCORES), np.float32)
    for m in range(NCORES):
        out[:, :, m] = res.results[m]["y"].T
    return out.reshape(1, VC, 8, 32, 32)



# revision 17
# speedup vs baseline: 1.3419x; 1.3419x over previous
"""Trainium2 Bass kernel for nn_AttentionBlock (causal single-head attention,
8192 tokens, qk-dim 16, v-dim 128, 1x1-conv projections with positional enc).

Sharding: striped query-parallel over 8 cores. Core m owns query tokens
{m, m+8, ..., m+8184} (1024 queries) — perfectly balanced causal work AND an
identical instruction stream on every core (required: one NEFF, SPMD). All
per-core variation is input data: the core's Q stripe and a tiny mask seed.

Device dataflow per core (bf16 data, fp32 PSUM accumulation):
  - Channels are contracted in 3 chunks (128+128+8) with bias folded in via a
    ones-channel; the 1/sqrt(259) score scale is folded into Wq/bq host-side.
  - Wq is replicated host-side into 4 column blocks at weight columns
    0/32/64/96, so the Q projection matmul emits qt already replicated at PE
    partition bases 0/32/64/96 for free (matmul cost depends only on the
    moving free dim). This lets K-projection chunks stack 4-per-PSUM-tile at
    those bases (one [128,512] copy moves 4 chunks instead of 4 copies) while
    satisfying the PE constraint that lhsT/rhs share a partition base.
  - V_aug [8192, 132] (tokens on partitions; col 128 is all-ones so softmax
    denominators come free from the attn@V matmul; cols 129-131 zero pad from
    zero weight columns). bf16 matmuls run 1 cycle/row at any free width.
  - The causal mask tiles are generated on device: a [128,512] column-index
    tensor (bf16, same on all cores) compared via tensor_scalar is_gt against
    a per-partition f32 seed (jp - m)/8 + 32*dloc. Replaces a 1 MB mask DMA.
  - Queries processed in PAIRS of 128-query subblocks: S^T tiles
    [128 keys, 256 q] (K stationary), exp on ScalarE (PSUM->SBUF, bf16 out),
    multiplicative causal mask (bf16 tensor_mul, DVE 2x) on the 16 diagonal
    key blocks of each pair, then attn@V accumulation with A^T stationary
    into one PSUM tile holding both subblocks (lo cols 0:132, hi 132:264).
  - Normalize with per-partition scale 1/(sum+1e-30) on DVE; DMA out as
    [1024 q, 128 vc] fp32; host reassembles/transposes stripes.
"""

import os
import numpy as np

P = 128
NTOK = 8192
KC, VC = 16, 128
NCORES = 8
NQ = NTOK // NCORES       # 1024 queries per core
QSUBS = NQ // P           # 8
NPAIR = QSUBS // 2        # 4 query-subblock pairs
VW = VC + 1               # 129: V columns + ones column
VSTR = 132                # V row stride in SBUF (128 v + 1 ones + zero pad)
CPAD = 264                # channels: 259 x + 3 pos + 1 ones + 1 zero pad
CTAIL = CPAD - 256        # rows in the third (partial) channel chunk
NGRP = NPAIR              # 4 DMA groups of 2048 tokens (4 chunks of 512)
WQR = 128                 # ww cols 0:128   = Wq replicated at bases 0/32/64/96
WKO = 128                 # ww cols 128:144 = Wk
WVO = 144                 # ww cols 144:276 = Wv | ones | zero pad

LAST_RESULTS = None       # BassKernelResults of the most recent run (for test.py)

_CACHE = {}


def _build_bass():
    import concourse.mybir as mybir
    import concourse.tile as tile
    from concourse import bacc

    f32 = mybir.dt.float32
    bf16 = mybir.dt.bfloat16
    AF = mybir.ActivationFunctionType
    ALU = mybir.AluOpType

    nc = bacc.Bacc("TRN2", target_bir_lowering=False, debug=False,
                   num_devices=NCORES)

    xq_d = nc.dram_tensor("xq", [CPAD, NQ], bf16, kind="ExternalInput").ap()
    xkv_d = nc.dram_tensor("xkv", [CPAD, NTOK], bf16, kind="ExternalInput").ap()
    ww_d = nc.dram_tensor("ww", [CPAD, 288], bf16, kind="ExternalInput").ap()
    colv_d = nc.dram_tensor("colv", [P, 512], bf16, kind="ExternalInput").ap()
    seed_d = nc.dram_tensor("seed", [P, 1], f32, kind="ExternalInput").ap()
    y_d = nc.dram_tensor("y", [NQ, VC], f32, kind="ExternalOutput").ap()
    dbg = bool(int(os.environ.get("KDBG", "0")))
    if dbg:
        bf = mybir.dt.bfloat16
        qtd = nc.dram_tensor("qtd", [P, NQ], bf, kind="ExternalOutput").ap()
        ktd = nc.dram_tensor("ktd", [P, NGRP * 512], bf, kind="ExternalOutput").ap()
        vd = nc.dram_tensor("vd", [P, 16 * VSTR], bf, kind="ExternalOutput").ap()
        ad = nc.dram_tensor("ad", [P, 8 * 512], bf, kind="ExternalOutput").ap()
        md = nc.dram_tensor("md", [P, 8 * 512], bf, kind="ExternalOutput").ap()

    with tile.TileContext(nc) as tc:
        with (
            tc.tile_pool(name="const", bufs=1) as const,
            tc.tile_pool(name="xpool", bufs=3) as xpool,
            tc.tile_pool(name="work", bufs=6) as work,
            tc.tile_pool(name="small", bufs=8) as small,
            tc.tile_pool(name="ps_s", bufs=3, space="PSUM") as ps_s,
            tc.tile_pool(name="ps_o", bufs=1, space="PSUM") as ps_o,
            tc.tile_pool(name="ps_kq", bufs=1, space="PSUM") as ps_kq,
            tc.tile_pool(name="ps_v", bufs=2, space="PSUM") as ps_v,
        ):
          # ---- body (emitted KREPEAT times for device-time measurement) ----
          for _rep in range(int(os.environ.get("KREPEAT", "1"))):
            # ---- persistent SBUF tensors ----
            ww_sb = const.tile([P, 3, 288], bf16)
            xq_sb = const.tile([P, 3, NQ], bf16)
            colv_sb = const.tile([P, 512], bf16)
            seed_sb = const.tile([P, 1], f32)
            mask_sb = const.tile([P, 8, 512], bf16)
            qt_sb = const.tile([P, NQ], bf16)
            kt_sb = const.tile([P, NGRP, 512], bf16)
            v_sb = const.tile([P, (NTOK // P) * VSTR], bf16)

            # ---- DMA ordering: weights and Q input first (Q projection and
            # its PSUM->SBUF copies are on pair-0's critical path), then the
            # K/V token groups in need order ----
            nc.sync.dma_start(ww_sb[:, 0:2, :],
                              ww_d[0:256].rearrange("(c p) m -> p c m", p=P))
            nc.sync.dma_start(ww_sb[0:CTAIL, 2, :], ww_d[256:CPAD])
            nc.sync.dma_start(xq_sb[:, 0:2, :],
                              xq_d[0:256].rearrange("(c p) n -> p c n", p=P))
            nc.sync.dma_start(xq_sb[0:CTAIL, 2, :], xq_d[256:CPAD])
            nc.sync.dma_start(colv_sb[:], colv_d)
            nc.sync.dma_start(seed_sb[:], seed_d)
            wq_sb = ww_sb[:, :, 0:WQR]
            wk_sb = ww_sb[:, :, WKO:WKO + KC]
            wv_sb = ww_sb[:, :, WVO:WVO + VSTR]
            grp_tiles = {}

            def ensure_group_dma(g):
                if g in grp_tiles or g >= NGRP:
                    return
                xg = xpool.tile([P, 3, 2048], bf16, tag="xg", name=f"xg{g}")
                halves = 2 if g == 0 else 1
                for hh in range(halves):
                    w = 2048 // halves
                    sl = slice(2048 * g + w * hh, 2048 * g + w * (hh + 1))
                    cs = slice(w * hh, w * (hh + 1))
                    nc.sync.dma_start(
                        xg[:, 0:2, cs],
                        xkv_d[0:256, sl].rearrange("(c p) n -> p c n", p=P))
                    nc.sync.dma_start(xg[0:CTAIL, 2, cs], xkv_d[256:CPAD, sl])
                grp_tiles[g] = xg

            ensure_group_dma(0)
            ensure_group_dma(1)

            # ---- causal mask tiles, generated on device (DVE is idle while
            # the input DMAs stream in) ----
            for dloc in range(8):
                rv = small.tile([P, 1], f32, tag="recip")
                nc.vector.tensor_scalar_add(rv[:], seed_sb[:], 32.0 * dloc)
                nc.vector.tensor_scalar(mask_sb[:, dloc, :], colv_sb[:],
                                        rv[:], None, ALU.is_gt)

            # ---- Q projection: qt [16, 1024] replicated at partition bases
            # 0/32/64/96 (Wq is host-replicated into 4 column blocks) ----
            for h in range(2):
                q_ps = ps_s.tile([P, 512], f32, tag="s")
                for ci in range(3):
                    kk = P if ci < 2 else CTAIL
                    nc.tensor.matmul(
                        q_ps[:], wq_sb[0:kk, ci, :],
                        xq_sb[0:kk, ci, h * 512:(h + 1) * 512],
                        start=(ci == 0), stop=(ci == 2),
                    )
                nc.vector.tensor_copy(qt_sb[:, h * 512:(h + 1) * 512], q_ps[:])

            def emit_k_chunk(kq, p, j):
                """K-projection matmuls for chunk 4p+j at partition base 32j."""
                xg = grp_tiles[p]
                for ci in range(3):
                    kk = P if ci < 2 else CTAIL
                    nc.tensor.matmul(
                        kq[32 * j:32 * j + KC, :], wk_sb[0:kk, ci, :],
                        xg[0:kk, ci, j * 512:(j + 1) * 512],
                        start=(ci == 0), stop=(ci == 2),
                        tile_position=(0, 32 * j),
                    )

            def emit_v_chunk(p, j):
                """V-projection for the 4 token tiles of chunk 4p+j."""
                xg = grp_tiles[p]
                for tl in range(4):
                    tb = 16 * p + 4 * j + tl
                    if tl % 2 == 0:
                        v_ps = ps_v.tile([P, 2 * VSTR], f32, tag="v")
                    col = VSTR * (tl % 2)
                    for ci in range(3):
                        kk = P if ci < 2 else CTAIL
                        nc.tensor.matmul(
                            v_ps[:, col:col + VSTR],
                            xg[0:kk, ci, j * 512 + tl * P:j * 512 + (tl + 1) * P],
                            wv_sb[0:kk, ci, :],
                            start=(ci == 0), stop=(ci == 2),
                        )
                    if tl % 2 == 1:
                        # (GPSIMD cannot read PSUM, so these stay on DVE)
                        dst = v_sb[:, (tb - 1) * VSTR:(tb + 1) * VSTR]
                        nc.vector.tensor_copy(dst, v_ps[:])

            # ---- main loop over query-subblock pairs ----
            for p in range(NPAIR):
                ensure_group_dma(p + 1)
                ensure_group_dma(p + 2)

                kq = ps_kq.tile([P, 512], f32, tag="kq")
                if p == 0:
                    # fine-grained start-up: copy each K chunk as it lands so
                    # S tiles can start before the whole group is projected
                    for j in range(2):
                        emit_k_chunk(kq, p, j)
                        nc.vector.tensor_copy(
                            kt_sb[32 * j:32 * j + KC, p, :],
                            kq[32 * j:32 * j + KC, :])
                    emit_v_chunk(p, 0)
                    for j in range(2, 4):
                        emit_k_chunk(kq, p, j)
                        nc.vector.tensor_copy(
                            kt_sb[32 * j:32 * j + KC, p, :],
                            kq[32 * j:32 * j + KC, :])
                    for j in range(1, 4):
                        emit_v_chunk(p, j)
                else:
                    for j in range(4):
                        emit_k_chunk(kq, p, j)
                    nc.vector.tensor_copy(kt_sb[:, p, :], kq[:])
                    for j in range(4):
                        emit_v_chunk(p, j)
                grp_tiles.pop(p)

                # attention for this pair
                nkb = 16 * p + 16          # key blocks for this pair
                ngr = nkb // 2             # S-tile groups (2 kbs each)
                nmm_lo = 16 * p + 8
                # lo and hi accumulators live in separate PSUM banks:
                # accumulation groups that interleave within one bank clobber
                # each other (a later start=True wipes the sibling's region)
                o_t = ps_o.tile([P, 2, 512], f32, tag="o")
                for g in range(ngr):
                    s_ps = ps_s.tile([P, 512], f32, tag="s")
                    for u in range(2):
                        kb = 2 * g + u
                        c = kb // 4
                        b = 32 * (c % 4)
                        klhs = kt_sb[b:b + KC, c // 4,
                                     128 * (kb % 4):128 * (kb % 4) + 128]
                        qrhs = qt_sb[b:b + KC, 256 * p:256 * p + 256]
                        nc.tensor.matmul(
                            s_ps[:, u * 256:(u + 1) * 256], klhs, qrhs,
                            start=True, stop=True,
                            tile_position=(b, 0),
                        )
                    a_sb = work.tile([P, 512], bf16, tag="a")
                    nc.scalar.activation(a_sb[:], s_ps[:], AF.Exp)
                    if g >= ngr - 8:
                        dloc = g - (ngr - 8)
                        nc.vector.tensor_mul(a_sb[:], a_sb[:],
                                             mask_sb[:, dloc, :])
                    if dbg and p == 0:
                        nc.sync.dma_start(ad[:, g * 512:(g + 1) * 512], a_sb[:])
                    for u in range(2):
                        kb = 2 * g + u
                        vrhs = v_sb[:, kb * VSTR:kb * VSTR + VSTR]
                        # lower qsub of the pair: only kbs strictly below its
                        # diagonal region contribute
                        if kb < nmm_lo:
                            nc.tensor.matmul(
                                o_t[:, 0, 0:VSTR],
                                a_sb[:, u * 256:u * 256 + P], vrhs,
                                start=(kb == 0), stop=(kb == nmm_lo - 1),
                            )
                        nc.tensor.matmul(
                            o_t[:, 1, 0:VSTR],
                            a_sb[:, u * 256 + P:u * 256 + 256], vrhs,
                            start=(kb == 0), stop=(kb == nkb - 1),
                        )

                if dbg and p == 0:
                    nc.sync.dma_start(qtd, qt_sb[:])
                    nc.sync.dma_start(
                        ktd, kt_sb[:].rearrange("p a b -> p (a b)"))
                    nc.sync.dma_start(vd, v_sb[:, 0:16 * VSTR])
                    nc.sync.dma_start(
                        md, mask_sb[:].rearrange("p a b -> p (a b)"))

                for half in range(2):
                    qs = 2 * p + half
                    recip_sb = small.tile([P, 1], f32, tag="recip")
                    nc.vector.tensor_scalar_add(
                        recip_sb[:], o_t[:, half, VC:VC + 1], 1e-30)
                    nc.vector.reciprocal(recip_sb[:], recip_sb[:])
                    on_sb = small.tile([P, VC], f32, tag="on")
                    nc.vector.tensor_scalar_mul(
                        on_sb[:], o_t[:, half, 0:VC], recip_sb[:])
                    nc.sync.dma_start(y_d[qs * P:(qs + 1) * P, :], on_sb[:])

    nc.compile()
    return nc


def _host_prep(x, Wq, bq, Wk, bk, Wv, bv):
    import ml_dtypes
    x = np.ascontiguousarray(np.asarray(x, np.float32))
    xc = np.zeros((CPAD, NTOK), np.float32)
    xc[:259] = x.reshape(259, NTOK)
    t = np.arange(8, dtype=np.float32) / 8 - 0.5
    h = np.arange(32, dtype=np.float32) / 32 - 0.5
    w = np.arange(32, dtype=np.float32) / 32 - 0.5
    pe = np.zeros((3, 8, 32, 32), np.float32)
    pe[0] = t[:, None, None]
    pe[1] = h[None, :, None]
    pe[2] = w[None, None, :]
    xc[259:262] = pe.reshape(3, NTOK)
    xc[262] = 1.0
    s = np.float32(1.0) / np.sqrt(np.float32(259.0))
    ww = np.zeros((CPAD, 288), np.float32)
    for b in range(4):
        ww[:262, 32 * b:32 * b + KC] = np.asarray(Wq, np.float32).T * s
        ww[262, 32 * b:32 * b + KC] = np.asarray(bq, np.float32) * s
    ww[:262, WKO:WKO + KC] = np.asarray(Wk, np.float32).T
    ww[262, WKO:WKO + KC] = np.asarray(bk, np.float32)
    ww[:262, WVO:WVO + VC] = np.asarray(Wv, np.float32).T
    ww[262, WVO:WVO + VC] = np.asarray(bv, np.float32)
    ww[262, WVO + VC] = 1.0
    return (xc.astype(ml_dtypes.bfloat16),
            ww.astype(ml_dtypes.bfloat16))


def _mask_aux(m):
    import ml_dtypes
    col = np.arange(512)
    colv = (col % 256 - 16 * (col // 256)).astype(np.float32)
    colv = np.broadcast_to(colv[None, :], (P, 512))
    seed = ((np.arange(P) - m) / 8.0).astype(np.float32)[:, None]
    return (np.ascontiguousarray(colv.astype(ml_dtypes.bfloat16)),
            np.ascontiguousarray(seed))


def kernel(x, Wq, bq, Wk, bk, Wv, bv):
    global LAST_RESULTS
    from concourse.bass_utils import run_bass_kernel_spmd

    if "nc" not in _CACHE:
        _CACHE["nc"] = _build_bass()
    nc = _CACHE["nc"]

    xc, ww = _host_prep(x, Wq, bq, Wk, bk, Wv, bv)
    in_maps = []
    for m in range(NCORES):
        colv, seed = _mask_aux(m)
        in_maps.append({
            "xq": np.ascontiguousarray(xc[:, m::8]),
            "xkv": xc,
            "ww": ww,
            "colv": colv,
            "seed": seed,
        })

    res = run_bass_kernel_spmd(
        nc, in_maps, core_ids=list(range(NCORES)),
        trace=bool(int(os.environ.get("KBENCH_TRACE", "0"))),
    )
    LAST_RESULTS = res

    out = np.zeros((VC, NQ, NCORES), np.float32)
    for m in range(NCORES):
        out[:, :, m] = res.results[m]["y"].T
    return out.reshape(1, VC, 8, 32, 32)


# revision 21
# speedup vs baseline: 1.4551x; 1.0844x over previous
"""Trainium2 Bass kernel for nn_AttentionBlock (causal single-head attention,
8192 tokens, qk-dim 16, v-dim 128, 1x1-conv projections with positional enc).

Sharding: striped query-parallel over 8 cores. Core m owns query tokens
{m, m+8, ..., m+8184} (1024 queries) — perfectly balanced causal work AND an
identical instruction stream on every core (required: one NEFF, SPMD). All
per-core variation is input data: the core's Q stripe and a tiny mask seed.

Device dataflow per core (bf16 data, fp32 PSUM accumulation):
  - Channels are contracted in 3 chunks (128+128+8) with bias folded in via a
    ones-channel; the 1/sqrt(259) score scale is folded into Wq/bq host-side.
  - Wq is replicated host-side into 4 column blocks at weight columns
    0/32/64/96, so the Q projection matmul emits qt already replicated at PE
    partition bases 0/32/64/96 for free (matmul cost depends only on the
    moving free dim). This lets K-projection chunks stack 4-per-PSUM-tile at
    those bases (one [128,512] copy moves 4 chunks instead of 4 copies) while
    satisfying the PE constraint that lhsT/rhs share a partition base.
  - V_aug [8192, 132] (tokens on partitions; col 128 is all-ones so softmax
    denominators come free from the attn@V matmul; cols 129-131 zero pad from
    zero weight columns). bf16 matmuls run 1 cycle/row at any free width.
  - The causal mask tiles are generated on device: a [128,512] column-index
    tensor (bf16, same on all cores) compared via tensor_scalar is_gt against
    a per-partition f32 seed (jp - m)/8 + 32*dloc. Replaces a 1 MB mask DMA.
  - Queries processed in PAIRS of 128-query subblocks: S^T tiles
    [128 keys, 256 q] (K stationary), exp on ScalarE (PSUM->SBUF, bf16 out),
    multiplicative causal mask (bf16 tensor_mul, DVE 2x) on the 16 diagonal
    key blocks of each pair, then attn@V accumulation with A^T stationary
    into one PSUM tile holding both subblocks (lo cols 0:132, hi 132:264).
  - Normalize with per-partition scale 1/(sum+1e-30) on DVE; DMA out as
    [1024 q, 128 vc] fp32; host reassembles/transposes stripes.
"""

import os
import numpy as np

P = 128
NTOK = 8192
KC, VC = 16, 128
NCORES = 8
NQ = NTOK // NCORES       # 1024 queries per core
QSUBS = NQ // P           # 8
NPAIR = QSUBS // 2        # 4 query-subblock pairs
VW = VC + 1               # 129: V columns + ones column
VSTR = 132                # V row stride in SBUF (128 v + 1 ones + zero pad)
CPAD = 264                # channels: 259 x + 3 pos + 1 ones + 1 zero pad
CTAIL = CPAD - 256        # rows in the third (partial) channel chunk
NGRP = NPAIR              # 4 DMA groups of 2048 tokens (4 chunks of 512)
WQR = 128                 # ww cols 0:128   = Wq replicated at bases 0/32/64/96
WKO = 128                 # ww cols 128:144 = Wk
WVO = 144                 # ww cols 144:276 = Wv | ones | zero pad

LAST_RESULTS = None       # BassKernelResults of the most recent run (for test.py)

_CACHE = {}


def _build_bass():
    import concourse.mybir as mybir
    import concourse.tile as tile
    from concourse import bacc

    f32 = mybir.dt.float32
    bf16 = mybir.dt.bfloat16
    AF = mybir.ActivationFunctionType
    ALU = mybir.AluOpType

    nc = bacc.Bacc("TRN2", target_bir_lowering=False, debug=False,
                   num_devices=NCORES)

    xq_d = nc.dram_tensor("xq", [CPAD, NQ], bf16, kind="ExternalInput").ap()
    xkv_d = nc.dram_tensor("xkv", [CPAD, NTOK], bf16, kind="ExternalInput").ap()
    ww_d = nc.dram_tensor("ww", [CPAD, 288], bf16, kind="ExternalInput").ap()
    colv_d = nc.dram_tensor("colv", [P, 512], bf16, kind="ExternalInput").ap()
    seed_d = nc.dram_tensor("seed", [P, 1], f32, kind="ExternalInput").ap()
    y_d = nc.dram_tensor("y", [NQ, VC], f32, kind="ExternalOutput").ap()

    with tile.TileContext(nc) as tc:
        with (
            tc.tile_pool(name="const", bufs=1) as const,
            tc.tile_pool(name="xpool", bufs=3) as xpool,
            tc.tile_pool(name="work", bufs=6) as work,
            tc.tile_pool(name="small", bufs=8) as small,
            tc.tile_pool(name="ps_s", bufs=3, space="PSUM") as ps_s,
            tc.tile_pool(name="ps_o", bufs=1, space="PSUM") as ps_o,
            tc.tile_pool(name="ps_kq", bufs=1, space="PSUM") as ps_kq,
            tc.tile_pool(name="ps_v", bufs=2, space="PSUM") as ps_v,
        ):
          # ---- body (emitted KREPEAT times for device-time measurement) ----
          for _rep in range(int(os.environ.get("KREPEAT", "1"))):
            # ---- persistent SBUF tensors ----
            ww_sb = const.tile([P, 3, 288], bf16)
            xq_sb = const.tile([P, 3, NQ], bf16)
            colv_sb = const.tile([P, 2, 256], bf16)
            seed_sb = const.tile([P, 1], f32)
            mask_sb = const.tile([P, 8, 2, 256], bf16)
            qt_sb = const.tile([P, NQ], bf16)
            kt_sb = const.tile([P, NGRP, 512], bf16)
            v_sb = const.tile([P, (NTOK // P) * VSTR], bf16)

            # ---- DMA ordering: weights and Q input first (Q projection and
            # its PSUM->SBUF copies are on pair-0's critical path), then the
            # K/V token groups in need order ----
            nc.sync.dma_start(ww_sb[:, 0:2, :],
                              ww_d[0:256].rearrange("(c p) m -> p c m", p=P))
            nc.sync.dma_start(ww_sb[0:CTAIL, 2, :], ww_d[256:CPAD])
            nc.sync.dma_start(xq_sb[:, 0:2, :],
                              xq_d[0:256].rearrange("(c p) n -> p c n", p=P))
            nc.sync.dma_start(xq_sb[0:CTAIL, 2, :], xq_d[256:CPAD])
            nc.sync.dma_start(colv_sb[:], colv_d)
            nc.sync.dma_start(seed_sb[:], seed_d)
            wq_sb = ww_sb[:, :, 0:WQR]
            wk_sb = ww_sb[:, :, WKO:WKO + KC]
            wv_sb = ww_sb[:, :, WVO:WVO + VSTR]
            grp_tiles = {}

            def ensure_group_dma(g):
                if g in grp_tiles or g >= NGRP:
                    return
                xg = xpool.tile([P, 3, 2048], bf16, tag="xg", name=f"xg{g}")
                halves = 2 if g == 0 else 1
                for hh in range(halves):
                    w = 2048 // halves
                    sl = slice(2048 * g + w * hh, 2048 * g + w * (hh + 1))
                    cs = slice(w * hh, w * (hh + 1))
                    nc.sync.dma_start(
                        xg[:, 0:2, cs],
                        xkv_d[0:256, sl].rearrange("(c p) n -> p c n", p=P))
                    nc.sync.dma_start(xg[0:CTAIL, 2, cs], xkv_d[256:CPAD, sl])
                grp_tiles[g] = xg

            ensure_group_dma(0)
            ensure_group_dma(1)

            # ---- causal mask tiles, generated on device (DVE is idle while
            # the input DMAs stream in) ----
            for dloc in range(8):
                rv = small.tile([P, 1], f32, tag="recip")
                nc.vector.tensor_scalar_add(rv[:], seed_sb[:], 32.0 * dloc)
                nc.vector.tensor_scalar(mask_sb[:, dloc, :, :], colv_sb[:],
                                        rv[:], None, ALU.is_gt)

            # ---- Q projection: qt [16, 1024] replicated at partition bases
            # 0/32/64/96 (Wq is host-replicated into 4 column blocks) ----
            for h in range(2):
                q_ps = ps_s.tile([P, 512], f32, tag="s")
                for ci in range(3):
                    kk = P if ci < 2 else CTAIL
                    nc.tensor.matmul(
                        q_ps[:], wq_sb[0:kk, ci, :],
                        xq_sb[0:kk, ci, h * 512:(h + 1) * 512],
                        start=(ci == 0), stop=(ci == 2),
                    )
                nc.vector.tensor_copy(qt_sb[:, h * 512:(h + 1) * 512], q_ps[:])

            def emit_k_chunk(kq, p, j):
                """K-projection matmuls for chunk 4p+j at partition base 32j."""
                xg = grp_tiles[p]
                for ci in range(3):
                    kk = P if ci < 2 else CTAIL
                    nc.tensor.matmul(
                        kq[32 * j:32 * j + KC, :], wk_sb[0:kk, ci, :],
                        xg[0:kk, ci, j * 512:(j + 1) * 512],
                        start=(ci == 0), stop=(ci == 2),
                        tile_position=(0, 32 * j),
                    )

            def emit_v_chunk(p, j):
                """V-projection for the 4 token tiles of chunk 4p+j."""
                xg = grp_tiles[p]
                for tl in range(4):
                    tb = 16 * p + 4 * j + tl
                    if tl % 2 == 0:
                        v_ps = ps_v.tile([P, 2 * VSTR], f32, tag="v")
                    col = VSTR * (tl % 2)
                    for ci in range(3):
                        kk = P if ci < 2 else CTAIL
                        nc.tensor.matmul(
                            v_ps[:, col:col + VSTR],
                            xg[0:kk, ci, j * 512 + tl * P:j * 512 + (tl + 1) * P],
                            wv_sb[0:kk, ci, :],
                            start=(ci == 0), stop=(ci == 2),
                        )
                    if tl % 2 == 1:
                        # (GPSIMD cannot read PSUM, so these stay on DVE)
                        dst = v_sb[:, (tb - 1) * VSTR:(tb + 1) * VSTR]
                        nc.vector.tensor_copy(dst, v_ps[:])

            def emit_attn_group(p, g, o_t):
                """One S-tile group (2 key blocks x 256 queries): S matmuls,
                exp, causal mask, attn@V accumulation.

                Diagonal groups (the last 8 of a pair) restrict the query
                columns: for group dloc, columns < 32*dloc are fully masked
                for BOTH of its key blocks, so the S matmul / exp / mask skip
                them entirely. attn@V still reads the full 128-query lhsT
                block, so the skipped region that an accumulation reads is
                zeroed with a cheap GPSIMD memset instead.
                """
                nkb = 16 * p + 16
                ngr = nkb // 2
                nmm_lo = 16 * p + 8
                diag = g >= ngr - 8
                dloc = g - (ngr - 8)
                cut = 32 * dloc if diag else 0
                s_ps = ps_s.tile([P, 2, 256], f32, tag="s")
                for u in range(2):
                    kb = 2 * g + u
                    c = kb // 4
                    b = 32 * (c % 4)
                    klhs = kt_sb[b:b + KC, c // 4,
                                 128 * (kb % 4):128 * (kb % 4) + 128]
                    qrhs = qt_sb[b:b + KC, 256 * p + cut:256 * p + 256]
                    nc.tensor.matmul(
                        s_ps[:, u, cut:256], klhs, qrhs,
                        start=True, stop=True,
                        tile_position=(b, 0),
                    )
                a_sb = work.tile([P, 2, 256], bf16, tag="a")
                if diag and 0 < cut <= 96:
                    # lo attn@V reads cols [0:128): zero the skipped region
                    nc.gpsimd.memset(a_sb[:, :, 0:cut], 0.0)
                elif diag and cut > 128:
                    # hi attn@V reads cols [128:256): zero its skipped region
                    nc.gpsimd.memset(a_sb[:, :, 128:cut], 0.0)
                nc.scalar.activation(a_sb[:, :, cut:256], s_ps[:, :, cut:256],
                                     AF.Exp)
                if diag:
                    nc.vector.tensor_mul(a_sb[:, :, cut:256],
                                         a_sb[:, :, cut:256],
                                         mask_sb[:, dloc, :, cut:256])
                for u in range(2):
                    kb = 2 * g + u
                    vrhs = v_sb[:, kb * VSTR:kb * VSTR + VSTR]
                    # lower qsub of the pair: only kbs strictly below its
                    # diagonal region contribute
                    if kb < nmm_lo:
                        nc.tensor.matmul(
                            o_t[:, 0, 0:VSTR],
                            a_sb[:, u, 0:P], vrhs,
                            start=(kb == 0), stop=(kb == nmm_lo - 1),
                        )
                    nc.tensor.matmul(
                        o_t[:, 1, 0:VSTR],
                        a_sb[:, u, P:256], vrhs,
                        start=(kb == 0), stop=(kb == nkb - 1),
                    )

            def emit_norm(p, half, o_t):
                qs = 2 * p + half
                recip_sb = small.tile([P, 1], f32, tag="recip")
                nc.vector.tensor_scalar_add(
                    recip_sb[:], o_t[:, half, VC:VC + 1], 1e-30)
                nc.vector.reciprocal(recip_sb[:], recip_sb[:])
                on_sb = small.tile([P, VC], f32, tag="on")
                nc.vector.tensor_scalar_mul(
                    on_sb[:], o_t[:, half, 0:VC], recip_sb[:])
                nc.sync.dma_start(y_d[qs * P:(qs + 1) * P, :], on_sb[:])

            # ---- main loop over query-subblock pairs ----
            # lo and hi accumulators live in separate PSUM banks:
            # accumulation groups that interleave within one bank clobber
            # each other (a later start=True wipes the sibling's region)
            for p in range(NPAIR):
                ensure_group_dma(p + 1)
                ensure_group_dma(p + 2)
                ngr = 8 * p + 8
                o_t = ps_o.tile([P, 2, 512], f32, tag="o")

                if p == 0:
                    # start-up: everything is diagonal; pipeline per chunk so
                    # S tiles start as soon as chunk 0 is projected
                    kq = ps_kq.tile([P, 512], f32, tag="kq")
                    for j in range(4):
                        emit_k_chunk(kq, p, j)
                        nc.vector.tensor_copy(
                            kt_sb[32 * j:32 * j + KC, p, :],
                            kq[32 * j:32 * j + KC, :])
                        emit_v_chunk(p, j)
                        for g in (2 * j, 2 * j + 1):
                            emit_attn_group(p, g, o_t)
                else:
                    # off-diagonal S groups first (they only need K/V from
                    # earlier pairs) so ScalarE keeps draining exps while the
                    # PE runs this pair's K/V projection burst
                    for g in range(ngr - 8):
                        emit_attn_group(p, g, o_t)
                    kq = ps_kq.tile([P, 512], f32, tag="kq")
                    for j in range(4):
                        emit_k_chunk(kq, p, j)
                    nc.vector.tensor_copy(kt_sb[:, p, :], kq[:])
                    for j in range(4):
                        emit_v_chunk(p, j)
                    for g in range(ngr - 8, ngr):
                        emit_attn_group(p, g, o_t)
                grp_tiles.pop(p)
                emit_norm(p, 0, o_t)
                emit_norm(p, 1, o_t)

    nc.compile()
    return nc


def _host_prep(x, Wq, bq, Wk, bk, Wv, bv):
    import ml_dtypes
    x = np.ascontiguousarray(np.asarray(x, np.float32))
    xc = np.zeros((CPAD, NTOK), np.float32)
    xc[:259] = x.reshape(259, NTOK)
    t = np.arange(8, dtype=np.float32) / 8 - 0.5
    h = np.arange(32, dtype=np.float32) / 32 - 0.5
    w = np.arange(32, dtype=np.float32) / 32 - 0.5
    pe = np.zeros((3, 8, 32, 32), np.float32)
    pe[0] = t[:, None, None]
    pe[1] = h[None, :, None]
    pe[2] = w[None, None, :]
    xc[259:262] = pe.reshape(3, NTOK)
    xc[262] = 1.0
    s = np.float32(1.0) / np.sqrt(np.float32(259.0))
    ww = np.zeros((CPAD, 288), np.float32)
    for b in range(4):
        ww[:262, 32 * b:32 * b + KC] = np.asarray(Wq, np.float32).T * s
        ww[262, 32 * b:32 * b + KC] = np.asarray(bq, np.float32) * s
    ww[:262, WKO:WKO + KC] = np.asarray(Wk, np.float32).T
    ww[262, WKO:WKO + KC] = np.asarray(bk, np.float32)
    ww[:262, WVO:WVO + VC] = np.asarray(Wv, np.float32).T
    ww[262, WVO:WVO + VC] = np.asarray(bv, np.float32)
    ww[262, WVO + VC] = 1.0
    return (xc.astype(ml_dtypes.bfloat16),
            ww.astype(ml_dtypes.bfloat16))


def _mask_aux(m):
    import ml_dtypes
    col = np.arange(512)
    colv = (col % 256 - 16 * (col // 256)).astype(np.float32)
    colv = np.broadcast_to(colv[None, :], (P, 512))
    seed = ((np.arange(P) - m) / 8.0).astype(np.float32)[:, None]
    return (np.ascontiguousarray(colv.astype(ml_dtypes.bfloat16)),
            np.ascontiguousarray(seed))


def kernel(x, Wq, bq, Wk, bk, Wv, bv):
    global LAST_RESULTS
    from concourse.bass_utils import run_bass_kernel_spmd

    if "nc" not in _CACHE:
        _CACHE["nc"] = _build_bass()
    nc = _CACHE["nc"]

    xc, ww = _host_prep(x, Wq, bq, Wk, bk, Wv, bv)
    in_maps = []
    for m in range(NCORES):
        colv, seed = _mask_aux(m)
        in_maps.append({
            "xq": np.ascontiguousarray(xc[:, m::8]),
            "xkv": xc,
            "ww": ww,
            "colv": colv,
            "seed": seed,
        })

    res = run_bass_kernel_spmd(
        nc, in_maps, core_ids=list(range(NCORES)),
        trace=bool(int(os.environ.get("KBENCH_TRACE", "0"))),
    )
    LAST_RESULTS = res

    out = np.zeros((VC, NQ, NCORES), np.float32)
    for m in range(NCORES):
        out[:, :, m] = res.results[m]["y"].T
    return out.reshape(1, VC, 8, 32, 32)
